# revision 23
# baseline (speedup 1.0000x reference)
"""Dynamic per-sample 3D Gaussian blur on 8 NeuronCores.

Sharding: pure data parallelism over (batch=4) x (channel=2) -> 8 cores,
one [160,160,160] volume per core. Per core the separable blur runs as
three banded-matmul passes on the TensorEngine (conv axis contracted on
partitions); the stationary operand is the data chunk so each pass also
rotates the layout for the next axis:

  pass1 (D):  x0[d', (h,w)] -> psum[h-chunk, d]  @ fixed w -> x1[h', (w,d)]
  pass2 (H):  x1[h', (w,d)] -> psum[w-chunk, h]  @ fixed d -> x2[w', (d,h)]
  pass3 (W):  x2[w', (d,h)] -> psum[(d,h)-chunk, w] -> staging -> HBM

The 160-long conv axis is split 128+32; outputs whose 13-tap window
crosses the split accumulate a second small matmul into the same PSUM
bank (per-element has_written semantics). Three conv-axis-tail tensors
share one [128, S] tile at partition ranges [0:32)/[32:64)/[64:96).
PSUM tiles batch 3 chunks (480 f32 cols, one bank) so each psum->SBUF
copy amortizes the ~125-170ns PSUM access latency; copies alternate
between VectorE and ScalarE; the input f32->bf16 cast is spread over
GpSimd/Vector/Scalar. All copy destinations are contiguous.
Data and band matrices are bf16, PSUM fp32; gaussians are computed on
host in fp32 exactly as the reference.
"""

from contextlib import ExitStack

import numpy as np
import ml_dtypes

import concourse.bass as bass
import concourse.tile as tile
from concourse import bacc, mybir
from concourse.bass_utils import run_bass_kernel_spmd

N = 160            # cube edge
S = N * N          # 25600 spatial positions per pass
NB = 13            # gaussian window
HALF = 6
A_N = 134          # big block out-cols [0, 134): windows within rows [0,128)
B_C0 = 122         # small block out-cols [122, 160): windows touching rows [128,160)
B_N = N - B_C0     # 38
GB_C0 = 3 * A_N    # col offset of the gb blocks in the packed G tile
G_COLS = 3 * A_N + B_N  # 440
EPS = 1e-7
GRP = 3            # psum chunks batched per bank (3*160 = 480 <= 512 f32)

BF16 = ml_dtypes.bfloat16
F32 = mybir.dt.float32
BF = mybir.dt.bfloat16

_PROGRAM = None


def _gaussian_1d(sigma):
    # fp32 replica of reference._gaussian_1d for a single sigma
    loc = (np.arange(NB, dtype=np.float32) - np.float32((NB - 1) / 2.0))
    s = np.float32(sigma)
    g = np.exp(-(loc * loc) / (2.0 * s * s + np.float32(EPS))
               - np.log(np.sqrt(np.float32(2.0 * np.pi)) * s + np.float32(EPS)))
    g = g.astype(np.float32)
    return g / g.sum(dtype=np.float32)


def _band(g):
    # T[r, c] = g[r - c + HALF] on the band, zero elsewhere ('SAME' zero pad)
    t = np.zeros((N, N), np.float32)
    for k in range(NB):
        off = k - HALF  # r = c + off
        c0 = max(0, -off)
        c1 = min(N, N - off)
        idx = np.arange(c0, c1)
        t[idx + off, idx] = g[k]
    return t


def _gpack(sigma_row):
    """[128, G_COLS] bf16: cols [p*134,(p+1)*134) = T_p[0:128, 0:134];
    cols [402:440) partitions [32p, 32p+32) = T_p[128:160, 122:160)."""
    out = np.zeros((128, G_COLS), np.float32)
    for p in range(3):
        t = _band(_gaussian_1d(sigma_row[p]))
        out[:, p * A_N:(p + 1) * A_N] = t[0:128, 0:A_N]
        out[32 * p:32 * (p + 1), GB_C0:G_COLS] = t[128:N, B_C0:N]
    return out.astype(BF16)


def _build_kernel(ctx, tc, x_in, g_in, y_out):
    nc = tc.nc

    gpool = ctx.enter_context(tc.tile_pool(name="g", bufs=1))
    big = ctx.enter_context(tc.tile_pool(name="big", bufs=1))
    tmp32 = ctx.enter_context(tc.tile_pool(name="tmp32", bufs=3))
    ps1 = ctx.enter_context(tc.tile_pool(name="ps1", bufs=4, space="PSUM"))
    ps2 = ctx.enter_context(tc.tile_pool(name="ps2", bufs=4, space="PSUM"))

    gtile = gpool.tile([128, G_COLS], BF)
    nc.sync.dma_start(gtile[:], g_in)

    def ga(p):  # [128, 134] base partition 0
        return gtile[:, (p - 1) * A_N:p * A_N]

    def gb(p):  # [32, 38] base partition 32*(p-1)
        return gtile[32 * (p - 1):32 * p, GB_C0:G_COLS]

    # persistent volume tiles; tails: [0:32) x0, [32:64) x1, [64:96) x2
    x0t1 = big.tile([128, S], BF, tag="sA")
    x1t1 = big.tile([128, S], BF, tag="sB")
    tails = big.tile([128, S], BF, tag="sT")

    # ---- load + cast input (f32 slabs -> bf16) on DVE + ACT
    SLAB = 2048
    for idx, c0 in enumerate(range(0, S, SLAB)):
        sl = min(SLAB, S - c0)
        sa = tmp32.tile([128, SLAB], F32, tag="t")
        nc.sync.dma_start(sa[0:128, 0:sl], x_in[0:128, c0:c0 + sl])
        if idx % 2 == 0:
            nc.vector.tensor_copy(x0t1[:, c0:c0 + sl], sa[0:128, 0:sl])
        else:
            nc.scalar.copy(x0t1[:, c0:c0 + sl], sa[0:128, 0:sl])
        sb = tmp32.tile([32, SLAB], F32, tag="t")
        nc.sync.dma_start(sb[0:32, 0:sl], x_in[128:160, c0:c0 + sl])
        if idx % 2 == 0:
            nc.scalar.copy(tails[0:32, c0:c0 + sl], sb[0:32, 0:sl])
        else:
            nc.vector.tensor_copy(tails[0:32, c0:c0 + sl], sb[0:32, 0:sl])

    def conv_pass(p, groups):
        """groups: iterable of (glen, [(cols1, cols2)...], dst1, dst2, pb2)
        where cols1/cols2 select the stationary columns for each of the
        glen chunks; dst1 [128, glen*160] / dst2 [32, glen*160] are the
        contiguous SBUF destinations; pb2 is the psum/dst partition base
        for the 32-row tail chunks. All pa matmuls are issued together,
        then all pb matmuls, so PE crosses one semaphore stream per psum
        tile; pa copies live on DVE, pb copies on ACT."""
        # mega-groups of 2 psum tiles: runs of up to 6 same-shape
        # stationaries so the PE weight ping-pong can pull LDWEIGHTS
        # ahead of in-flight matmuls; start=True clears the whole bank,
        # so only the first matmul per bank sets it
        groups = list(groups)
        for g0 in range(0, len(groups), 2):
            mg = groups[g0:g0 + 2]
            pas = [ps1.tile([128, GRP * N], F32, tag="pa", name=f"pa{g0}_{t}") for t in range(len(mg))]
            pbs = [ps2.tile([128, GRP * N], F32, tag="pb", name=f"pb{g0}_{t}") for t in range(len(mg))]
            for t, (glen, cols, dst1, dst2, pb2) in enumerate(mg):
                for j, (c1, _) in enumerate(cols):
                    nc.tensor.matmul(pas[t][0:128, j * N:j * N + A_N],
                                     c1[0], ga(p), start=(j == 0), stop=False,
                                     skip_group_check=True)
            for t, (glen, cols, dst1, dst2, pb2) in enumerate(mg):
                for j, (c1, _) in enumerate(cols):
                    o = j * N
                    nc.tensor.matmul(pas[t][0:128, o + B_C0:o + N],
                                     c1[1], gb(p), start=False,
                                     stop=(j == glen - 1),
                                     skip_group_check=True)
            for t, (glen, cols, dst1, dst2, pb2) in enumerate(mg):
                for j, (_, c2) in enumerate(cols):
                    nc.tensor.matmul(pbs[t][pb2:pb2 + 32, j * N:j * N + A_N],
                                     c2[0], ga(p), start=(j == 0), stop=False,
                                     skip_group_check=True)
            for t, (glen, cols, dst1, dst2, pb2) in enumerate(mg):
                for j, (_, c2) in enumerate(cols):
                    o = j * N
                    nc.tensor.matmul(pbs[t][pb2:pb2 + 32, o + B_C0:o + N],
                                     c2[1], gb(p), start=False,
                                     stop=(j == glen - 1),
                                     skip_group_check=True)
            for t, (glen, cols, dst1, dst2, pb2) in enumerate(mg):
                w = glen * N
                nc.vector.tensor_copy(dst1, pas[t][0:128, 0:w])
                nc.scalar.copy(dst2, pbs[t][pb2:pb2 + 32, 0:w])

    # ---- pass 1: conv along d; chunks = h values at fixed w (strided cols)
    x0v1 = x0t1[:].rearrange("p (h w) -> p h w", w=N)
    x0v2 = tails[0:32].rearrange("p (h w) -> p h w", w=N)

    def p1_groups():
        for w0 in range(0, N, GRP):
            glen = min(GRP, N - w0)
            cols = [((x0v1[:, 0:128, w0 + j], x0v2[:, 0:128, w0 + j]),
                     (x0v1[:, 128:160, w0 + j], x0v2[:, 128:160, w0 + j]))
                    for j in range(glen)]
            # x1 free layout (w, d): cols w*160+d -> w-group is contiguous
            yield (glen, cols,
                   x1t1[:, w0 * N:(w0 + glen) * N],
                   tails[32:64, w0 * N:(w0 + glen) * N], 32)

    conv_pass(1, p1_groups())

    # x2t1 reuses x0's slot (x0 fully consumed by pass 1)
    x2t1 = big.tile([128, S], BF, tag="sA")

    # ---- pass 2: conv along h; chunks = w values at fixed d (strided cols)
    x1v1 = x1t1[:].rearrange("p (w d) -> p w d", d=N)
    x1v2 = tails[32:64].rearrange("p (w d) -> p w d", d=N)

    def p2_groups():
        for d0 in range(0, N, GRP):
            glen = min(GRP, N - d0)
            cols = [((x1v1[:, 0:128, d0 + j], x1v2[:, 0:128, d0 + j]),
                     (x1v1[:, 128:160, d0 + j], x1v2[:, 128:160, d0 + j]))
                    for j in range(glen)]
            # x2 free layout (d, h): cols d*160+h -> d-group is contiguous
            yield (glen, cols,
                   x2t1[:, d0 * N:(d0 + glen) * N],
                   tails[64:96, d0 * N:(d0 + glen) * N], 64)

    conv_pass(2, p2_groups())

    # ---- pass 3: conv along w; chunks = contiguous (d,h) blocks; to HBM
    x2v2 = tails[64:96]
    yv = y_out.rearrange("(k p) w -> p k w", p=128)
    NK = S // 128  # 200
    k0s = list(range(0, NK, GRP))
    for gi in range(0, len(k0s), 2):
        mk = k0s[gi:gi + 2]
        pss = [ps1.tile([128, GRP * N], F32, tag="pa", name=f"p3_{gi}_{t}") for t in range(len(mk))]
        for t, k0 in enumerate(mk):
            glen = min(GRP, NK - k0)
            for j in range(glen):
                c = (k0 + j) * 128
                nc.tensor.matmul(pss[t][:, j * N:j * N + A_N],
                                 x2t1[:, c:c + 128], ga(3),
                                 start=(j == 0), stop=False,
                                 skip_group_check=True)
        for t, k0 in enumerate(mk):
            glen = min(GRP, NK - k0)
            for j in range(glen):
                o = j * N
                c = (k0 + j) * 128
                nc.tensor.matmul(pss[t][:, o + B_C0:o + N],
                                 x2v2[:, c:c + 128], gb(3),
                                 start=False, stop=(j == glen - 1),
                                 skip_group_check=True)
        for t, k0 in enumerate(mk):
            glen = min(GRP, NK - k0)
            st = tmp32.tile([128, GRP * N], F32, tag="t")
            if t == 0:
                nc.vector.tensor_copy(st[:, 0:glen * N], pss[t][:, 0:glen * N])
            else:
                nc.scalar.copy(st[:, 0:glen * N], pss[t][:, 0:glen * N])
            stv = st[:].rearrange("p (k w) -> p k w", w=N)
            nc.sync.dma_start(yv[0:128, k0:k0 + glen, :], stv[:, 0:glen, :])


def _build_program():
    global _PROGRAM
    if _PROGRAM is not None:
        return _PROGRAM
    nc = bacc.Bacc("TRN2", target_bir_lowering=False, debug=False,
                   num_devices=8)
    x_in = nc.dram_tensor("x_in", [N, S], F32, kind="ExternalInput").ap()
    g_in = nc.dram_tensor("g_in", [128, G_COLS], BF, kind="ExternalInput").ap()
    y_out = nc.dram_tensor("y_out", [S, N], F32, kind="ExternalOutput").ap()
    with tile.TileContext(nc) as tc, ExitStack() as ctx:
        _build_kernel(ctx, tc, x_in, g_in, y_out)
    nc.compile()
    _PROGRAM = nc
    return nc


def _run(image, sigma, **spmd_kwargs):
    nc = _build_program()
    B, _, _, _, C = image.shape
    in_maps = []
    for core in range(8):
        b, c = divmod(core, C)
        vol = np.ascontiguousarray(image[b, :, :, :, c]).reshape(N, S)
        in_maps.append({"x_in": vol, "g_in": _gpack(sigma[b])})
    res = run_bass_kernel_spmd(nc, in_maps, list(range(8)), **spmd_kwargs)
    out = np.empty((B, N, N, N, C), np.float32)
    for core in range(8):
        b, c = divmod(core, C)
        out[b, :, :, :, c] = res.results[core]["y_out"].reshape(N, N, N)
    return out, res


def kernel(image, sigma):
    image = np.asarray(image, dtype=np.float32)
    sigma = np.asarray(sigma, dtype=np.float32)
    out, _ = _run(image, sigma)
    return out


# revision 28
# speedup vs baseline: 1.1184x; 1.1184x over previous
"""Dynamic per-sample 3D Gaussian blur on 8 NeuronCores.

Sharding: pure data parallelism over (batch=4) x (channel=2) -> 8 cores,
one [160,160,160] volume per core. Per core the separable blur runs as
three banded-matmul passes on the TensorEngine (conv axis contracted on
partitions); the stationary operand is the data chunk so each pass also
rotates the layout for the next axis:

  pass1 (D):  x0[d', (h,w)] -> psum[h-chunk, d]  @ fixed w -> x1[h', (w,d)]
  pass2 (H):  x1[h', (w,d)] -> psum[w-chunk, h]  @ fixed d -> x2[w', (d,h)]
  pass3 (W):  x2[w', (d,h)] -> psum[(d,h)-chunk, w] -> staging -> HBM

The 160-long conv axis is split 128+32; outputs whose 13-tap window
crosses the split accumulate a second small matmul into the same PSUM
bank (per-element has_written semantics). Three conv-axis-tail tensors
share one [128, S] tile at partition ranges [0:32)/[32:64)/[64:96).
PSUM tiles batch 3 chunks (480 f32 cols, one bank) so each psum->SBUF
copy amortizes the ~125-170ns PSUM access latency; copies alternate
between VectorE and ScalarE; the input f32->bf16 cast is spread over
GpSimd/Vector/Scalar. All copy destinations are contiguous.
Data and band matrices are bf16, PSUM fp32; gaussians are computed on
host in fp32 exactly as the reference.
"""

from contextlib import ExitStack

import numpy as np
import ml_dtypes

import concourse.bass as bass
import concourse.tile as tile
from concourse import bacc, mybir
from concourse.bass_utils import run_bass_kernel_spmd

N = 160            # cube edge
S = N * N          # 25600 spatial positions per pass
NB = 13            # gaussian window
HALF = 6
A_N = 134          # big block out-cols [0, 134): windows within rows [0,128)
B_C0 = 122         # small block out-cols [122, 160): windows touching rows [128,160)
B_N = N - B_C0     # 38
GB_C0 = 3 * A_N    # col offset of the gb blocks in the packed G tile
G_COLS = 3 * A_N + B_N  # 440
EPS = 1e-7
GRP = 3            # psum chunks batched per bank (3*160 = 480 <= 512 f32)

BF16 = ml_dtypes.bfloat16
F32 = mybir.dt.float32
BF = mybir.dt.bfloat16

_PROGRAM = None


def _gaussian_1d(sigma):
    # fp32 replica of reference._gaussian_1d for a single sigma
    loc = (np.arange(NB, dtype=np.float32) - np.float32((NB - 1) / 2.0))
    s = np.float32(sigma)
    g = np.exp(-(loc * loc) / (2.0 * s * s + np.float32(EPS))
               - np.log(np.sqrt(np.float32(2.0 * np.pi)) * s + np.float32(EPS)))
    g = g.astype(np.float32)
    return g / g.sum(dtype=np.float32)


def _band(g):
    # T[r, c] = g[r - c + HALF] on the band, zero elsewhere ('SAME' zero pad)
    t = np.zeros((N, N), np.float32)
    for k in range(NB):
        off = k - HALF  # r = c + off
        c0 = max(0, -off)
        c1 = min(N, N - off)
        idx = np.arange(c0, c1)
        t[idx + off, idx] = g[k]
    return t


def _gpack(sigma_row):
    """[128, G_COLS] bf16: cols [p*134,(p+1)*134) = T_p[0:128, 0:134];
    cols [402:440) partitions [32p, 32p+32) = T_p[128:160, 122:160)."""
    out = np.zeros((128, G_COLS), np.float32)
    for p in range(3):
        t = _band(_gaussian_1d(sigma_row[p]))
        out[:, p * A_N:(p + 1) * A_N] = t[0:128, 0:A_N]
        out[32 * p:32 * (p + 1), GB_C0:G_COLS] = t[128:N, B_C0:N]
    return out.astype(BF16)


def _build_kernel(ctx, tc, x_in, g_in, y_out):
    nc = tc.nc

    gpool = ctx.enter_context(tc.tile_pool(name="g", bufs=1))
    big = ctx.enter_context(tc.tile_pool(name="big", bufs=1))
    tmp32 = ctx.enter_context(tc.tile_pool(name="tmp32", bufs=3))
    ps1 = ctx.enter_context(tc.tile_pool(name="ps1", bufs=6, space="PSUM"))
    ps2 = ctx.enter_context(tc.tile_pool(name="ps2", bufs=2, space="PSUM"))

    gtile = gpool.tile([128, G_COLS], BF)
    nc.sync.dma_start(gtile[:], g_in)

    def ga(p):  # [128, 134] base partition 0
        return gtile[:, (p - 1) * A_N:p * A_N]

    def gb(p):  # [32, 38] base partition 32*(p-1)
        return gtile[32 * (p - 1):32 * p, GB_C0:G_COLS]

    # persistent volume tiles; tails: [0:32) x0, [32:64) x1, [64:96) x2
    x0t1 = big.tile([128, S], BF, tag="sA")
    x1t1 = big.tile([128, S], BF, tag="sB")
    tails = big.tile([128, S], BF, tag="sT")

    # ---- load + cast input (f32 slabs -> bf16) on DVE + ACT; the d'-tail
    # rows go first so pass 1 (which needs all of them as MM_B stationary)
    # can start as soon as the main rows' cast finishes
    SLAB = 2048
    for idx, c0 in enumerate(range(0, S, SLAB)):
        sl = min(SLAB, S - c0)
        sb = tmp32.tile([32, SLAB], F32, tag="t")
        nc.sync.dma_start(sb[0:32, 0:sl], x_in[128:160, c0:c0 + sl])
        if idx % 2 == 0:
            nc.scalar.copy(tails[0:32, c0:c0 + sl], sb[0:32, 0:sl])
        else:
            nc.vector.tensor_copy(tails[0:32, c0:c0 + sl], sb[0:32, 0:sl])
    for idx, c0 in enumerate(range(0, S, SLAB)):
        sl = min(SLAB, S - c0)
        sa = tmp32.tile([128, SLAB], F32, tag="t")
        nc.sync.dma_start(sa[0:128, 0:sl], x_in[0:128, c0:c0 + sl])
        if idx % 2 == 0:
            nc.vector.tensor_copy(x0t1[:, c0:c0 + sl], sa[0:128, 0:sl])
        else:
            nc.scalar.copy(x0t1[:, c0:c0 + sl], sa[0:128, 0:sl])

    def conv_pass(p, groups):
        """groups: iterable of (glen, [(cols1, cols2)...], dst1, dst2, pb2)
        where cols1/cols2 select the stationary columns for each of the
        glen chunks; dst1 [128, glen*160] / dst2 [32, glen*160] are the
        contiguous SBUF destinations; pb2 is the psum/dst partition base
        for the 32-row tail chunks. All pa matmuls are issued together,
        then all pb matmuls, so PE crosses one semaphore stream per psum
        tile; pa copies live on DVE, pb copies on ACT."""
        # runs of same-shape stationaries so the PE weight ping-pong can
        # pull LDWEIGHTS ahead of in-flight matmuls; start=True clears
        # the whole bank, so only the first matmul per bank sets it
        for glen, cols, dst1, dst2, pb2 in groups:
            pa = ps1.tile([128, GRP * N], F32, tag="pa")
            pb = ps2.tile([128, GRP * N], F32, tag="pb")
            for j, (c1, _) in enumerate(cols):
                nc.tensor.matmul(pa[0:128, j * N:j * N + A_N], c1[0], ga(p),
                                 start=(j == 0), stop=False,
                                 skip_group_check=True)
            for j, (c1, _) in enumerate(cols):
                o = j * N
                nc.tensor.matmul(pa[0:128, o + B_C0:o + N], c1[1], gb(p),
                                 start=False, stop=(j == glen - 1),
                                 skip_group_check=True)
            for j, (_, c2) in enumerate(cols):
                nc.tensor.matmul(pb[pb2:pb2 + 32, j * N:j * N + A_N], c2[0], ga(p),
                                 start=(j == 0), stop=False,
                                 skip_group_check=True)
            for j, (_, c2) in enumerate(cols):
                o = j * N
                nc.tensor.matmul(pb[pb2:pb2 + 32, o + B_C0:o + N], c2[1], gb(p),
                                 start=False, stop=(j == glen - 1),
                                 skip_group_check=True)
            w = glen * N
            nc.vector.tensor_copy(dst1, pa[0:128, 0:w])
            nc.scalar.copy(dst2, pb[pb2:pb2 + 32, 0:w])

    # ---- pass 1: conv along d; chunks = h values at fixed w (strided cols)
    x0v1 = x0t1[:].rearrange("p (h w) -> p h w", w=N)
    x0v2 = tails[0:32].rearrange("p (h w) -> p h w", w=N)

    def p1_groups():
        for w0 in range(0, N, GRP):
            glen = min(GRP, N - w0)
            cols = [((x0v1[:, 0:128, w0 + j], x0v2[:, 0:128, w0 + j]),
                     (x0v1[:, 128:160, w0 + j], x0v2[:, 128:160, w0 + j]))
                    for j in range(glen)]
            # x1 free layout (w, d): cols w*160+d -> w-group is contiguous
            yield (glen, cols,
                   x1t1[:, w0 * N:(w0 + glen) * N],
                   tails[32:64, w0 * N:(w0 + glen) * N], 32)

    conv_pass(1, p1_groups())

    # x2t1 reuses x0's slot (x0 fully consumed by pass 1)
    x2t1 = big.tile([128, S], BF, tag="sA")

    # ---- pass 2: conv along h; chunks = w values at fixed d (strided cols)
    x1v1 = x1t1[:].rearrange("p (w d) -> p w d", d=N)
    x1v2 = tails[32:64].rearrange("p (w d) -> p w d", d=N)

    def p2_groups():
        for d0 in range(0, N, GRP):
            glen = min(GRP, N - d0)
            cols = [((x1v1[:, 0:128, d0 + j], x1v2[:, 0:128, d0 + j]),
                     (x1v1[:, 128:160, d0 + j], x1v2[:, 128:160, d0 + j]))
                    for j in range(glen)]
            # x2 free layout (d, h): cols d*160+h -> d-group is contiguous
            yield (glen, cols,
                   x2t1[:, d0 * N:(d0 + glen) * N],
                   tails[64:96, d0 * N:(d0 + glen) * N], 64)

    conv_pass(2, p2_groups())

    # ---- pass 3: conv along w; chunks = contiguous (d,h) blocks; to HBM
    x2v2 = tails[64:96]
    yv = y_out.rearrange("(k p) w -> p k w", p=128)
    NK = S // 128  # 200
    for k0 in range(0, NK, GRP):
        glen = min(GRP, NK - k0)
        ps = ps1.tile([128, GRP * N], F32, tag="pa")
        for j in range(glen):
            c = (k0 + j) * 128
            nc.tensor.matmul(ps[:, j * N:j * N + A_N], x2t1[:, c:c + 128], ga(3),
                             start=(j == 0), stop=False, skip_group_check=True)
        for j in range(glen):
            o = j * N
            c = (k0 + j) * 128
            nc.tensor.matmul(ps[:, o + B_C0:o + N], x2v2[:, c:c + 128], gb(3),
                             start=False, stop=(j == glen - 1),
                             skip_group_check=True)
        st = tmp32.tile([128, GRP * N], F32, tag="t")
        if (k0 // GRP) % 2 == 0:
            nc.vector.tensor_copy(st[:, 0:glen * N], ps[:, 0:glen * N])
        else:
            nc.scalar.copy(st[:, 0:glen * N], ps[:, 0:glen * N])
        stv = st[:].rearrange("p (k w) -> p k w", w=N)
        nc.sync.dma_start(yv[0:128, k0:k0 + glen, :], stv[:, 0:glen, :])


def _build_program():
    global _PROGRAM
    if _PROGRAM is not None:
        return _PROGRAM
    nc = bacc.Bacc("TRN2", target_bir_lowering=False, debug=False,
                   num_devices=8)
    x_in = nc.dram_tensor("x_in", [N, S], F32, kind="ExternalInput").ap()
    g_in = nc.dram_tensor("g_in", [128, G_COLS], BF, kind="ExternalInput").ap()
    y_out = nc.dram_tensor("y_out", [S, N], F32, kind="ExternalOutput").ap()
    with tile.TileContext(nc) as tc, ExitStack() as ctx:
        _build_kernel(ctx, tc, x_in, g_in, y_out)
    nc.compile()
    _PROGRAM = nc
    return nc


def _run(image, sigma, **spmd_kwargs):
    nc = _build_program()
    B, _, _, _, C = image.shape
    in_maps = []
    for core in range(8):
        b, c = divmod(core, C)
        vol = np.ascontiguousarray(image[b, :, :, :, c]).reshape(N, S)
        in_maps.append({"x_in": vol, "g_in": _gpack(sigma[b])})
    res = run_bass_kernel_spmd(nc, in_maps, list(range(8)), **spmd_kwargs)
    out = np.empty((B, N, N, N, C), np.float32)
    for core in range(8):
        b, c = divmod(core, C)
        out[b, :, :, :, c] = res.results[core]["y_out"].reshape(N, N, N)
    return out, res


def kernel(image, sigma):
    image = np.asarray(image, dtype=np.float32)
    sigma = np.asarray(sigma, dtype=np.float32)
    out, _ = _run(image, sigma)
    return out
